# revision 1
# baseline (speedup 1.0000x reference)
"""Trainium2 Bass kernel for EnhancedKANLayer (spline-order-3 KAN layer).

Reference computation (fp32):
    x_norm = tanh(x[:, None, :] / scaler[None, :, :])          # (B, O, I)
    d      = |x_norm[..., None] - grid|                        # (B, O, I, G)
    b      = exp(-d**3);  bhat = b / (sum_g b + 1e-8)
    out    = einsum('boig,oig->bo', bhat, W) + bias

With scaler uniform across O (as produced by setup_inputs: all-ones),
x_norm is O-independent, so the basis collapses to (B, I, G) and the
contraction becomes a (B, I*G) @ (I*G, O) matmul.

Strategy: data-parallel over batch across 8 NeuronCores (B=512 -> 64
rows/core, all params replicated; x/scaler folded on host). Per core,
raw-bacc program (manual semaphores, no Tile drain/barrier tail):
  ACT:    tanh, Square(v), Abs(v), Exp (one table set: exp_and_others,
          prefetched via a dummy exp), psum->sbuf copy
  DVE:    v = xn - g (8 tensor_scalars/half), g-reduce (quarters),
          1/S via reciprocal_approx_fast, normalize
  GPSIMD: weight DMAs, d3 = d2*|v| (quarters)
  PE:     K=1 bias matmul + 16 accumulating bf16 matmuls
          (K=128 chunks of I*G=2048, M=64, N=128) into one PSUM bank
Work is split into halves/quarters so the four engines pipeline.
Falls back to a pure-numpy reference path if scaler is not uniform
across O (never hit by the real input distribution).
"""

import os
import sys
import types

import numpy as np

N_CORES = 8
B, I, O, G = 512, 256, 128, 8
BS = B // N_CORES          # batch rows per core
NCH = I // 128             # i-chunks of 128 partitions
EPS = 1e-8

_CACHE = {}


def _ensure_axon_ntff_hook():
    """Register the NTFF profiling hook (missing antenv.axon_hooks shim).
    Only needed for traced runs; harmless otherwise."""
    try:
        import antenv
        if 'antenv.axon_hooks' not in sys.modules:
            mod = types.ModuleType('antenv.axon_hooks')
            holder = [None]
            mod.set_axon_ntff_profile_hook = lambda h: holder.__setitem__(0, h)
            mod.get_axon_ntff_profile_hook = lambda: holder[0]
            sys.modules['antenv.axon_hooks'] = mod
            antenv.axon_hooks = mod
        mod = sys.modules['antenv.axon_hooks']
        if mod.get_axon_ntff_profile_hook() is None:
            from trn_agent_boot.trn_boot import _ntff_profile_via_ctypes
            so = '/opt/axon/libaxon_pjrt.so'
            if os.path.exists(so):
                mod.set_axon_ntff_profile_hook(_ntff_profile_via_ctypes(so))
    except Exception:
        pass


def _reference_numpy(x, spline_weight, spline_scaler, bias, grid_points):
    """General fallback, mirrors the jax reference in numpy (fp32)."""
    x = x.astype(np.float32)
    xn = np.tanh(x[:, None, :] / spline_scaler[None, :, :])          # (B,O,I)
    d = np.abs(xn[..., None] - grid_points)                           # (B,O,I,G)
    b = np.exp(-(d ** 3))
    bhat = b / (b.sum(axis=-1, keepdims=True) + EPS)
    out = np.einsum('boig,oig->bo', bhat, spline_weight, optimize=True)
    return (out + bias[None, :]).astype(np.float32)


def _build_program_raw(grid_vals, mm_bf16=True):
    """Raw bacc (no TileContext): manual semaphores, no drain/barrier tail.

    Engine plan per core (B-shard=64 rows):
      SYNC:   misc DMA in, out DMA
      GPSIMD: weight DMAs, d3 = d2*|v| multiplies
      ACT:    table-prefetch dummy, tanh, Square, Abs, Exp, psum1 copy
      DVE:    ones memset, v = xn-g, g-reduce, 1/S, normalize, final add
      PE:     16 accumulating bf16 matmuls + K=1 bias matmul
    Work is split into two halves (i-chunks) so ACT/DVE/GPSIMD pipeline.
    """
    from contextlib import ExitStack

    import concourse.bass as bass
    from concourse import bacc, mybir

    f32 = mybir.dt.float32
    bf16 = mybir.dt.bfloat16
    AF = mybir.ActivationFunctionType
    ALU = mybir.AluOpType

    nc = bacc.Bacc("TRN2", target_bir_lowering=False, debug=False,
                   num_devices=N_CORES)

    FQ = NCH * BS            # 128: packed free dim (ch, b)
    FB = G * FQ              # 1024: packed basis free dim (g, ch, b)
    MC = FQ + O + G          # misc cols: xT | bias(row0) | grid(all rows)
    misc_d = nc.dram_tensor("misc", [128, MC], f32, kind="ExternalInput")
    mmdt = bf16 if mm_bf16 else f32
    wp_d = nc.dram_tensor("wp", [128, NCH * G * O], mmdt,
                          kind="ExternalInput")
    out_d = nc.dram_tensor("out", [BS, O], f32, kind="ExternalOutput")
    wc = NCH * G * O // 2

    with ExitStack() as ctx:
        e = ctx.enter_context
        misc = e(nc.sbuf_tensor([128, MC], f32))
        wp = e(nc.sbuf_tensor([128, NCH * G * O], mmdt))
        ones = e(nc.sbuf_tensor([1, BS], f32))
        dummy = e(nc.sbuf_tensor([1, 8], f32))
        xn = e(nc.sbuf_tensor([128, FQ], f32))
        v = e(nc.sbuf_tensor([128, FB], f32))
        d2 = e(nc.sbuf_tensor([128, FB], f32))
        a = e(nc.sbuf_tensor([128, FB], f32))
        d3 = e(nc.sbuf_tensor([128, FB], f32))
        E = e(nc.sbuf_tensor([128, FB], f32))
        S = e(nc.sbuf_tensor([128, FQ], f32))
        r = e(nc.sbuf_tensor([128, FQ], f32))
        En = e(nc.sbuf_tensor([128, FB], mmdt))
        outsb = e(nc.sbuf_tensor([BS, O], f32))
        wsrc = e(nc.sbuf_tensor([128, O], f32))
        psum0 = e(nc.psum_tensor([BS, O], f32))
        scr_ps = e(nc.psum_tensor([BS, O], f32))

        dmaM = e(nc.semaphore("dmaM"))
        dmaM2 = e(nc.semaphore("dmaM2"))
        dmaW = e(nc.semaphore("dmaW"))
        dmaO = e(nc.semaphore("dmaO"))
        sOnes = e(nc.semaphore("sOnes"))
        sA = e(nc.semaphore("sA"))
        sV = e(nc.semaphore("sV"))
        sQ = e(nc.semaphore("sQ"))
        sD = e(nc.semaphore("sD"))
        sE = e(nc.semaphore("sE"))
        sN = e(nc.semaphore("sN"))
        sP0 = e(nc.semaphore("sP0"))
        sC = e(nc.semaphore("sC"))

        block = e(nc.Block(no_gpsimd_drain=True))

        xt = misc[:, 0:FQ]
        bias_row = misc[0:1, FQ:FQ + O]
        grid_t = misc[:, FQ + O:FQ + O + G]

        v3 = v[:].rearrange("p (g q) -> p g q", q=FQ)
        d23 = d2[:].rearrange("p (g q) -> p g q", q=FQ)
        a3 = a[:].rearrange("p (g q) -> p g q", q=FQ)
        d33 = d3[:].rearrange("p (g q) -> p g q", q=FQ)
        E3 = E[:].rearrange("p (g q) -> p g q", q=FQ)
        E4 = E[:].rearrange("p (g q) -> p q g", q=FQ)
        En3 = En[:].rearrange("p (g q) -> p g q", q=FQ)

        def qs(h):
            return slice(h * BS, (h + 1) * BS)

        @block.sync
        def _(sync):
            sync.dma_start(misc[:, 0:BS], misc_d.ap()[:, 0:BS]).then_inc(dmaM, 16)
            sync.wait_ge(sC, 1)
            sync.dma_start(out_d.ap(), outsb[:]).then_inc(dmaO, 16)
            sync.wait_ge(dmaO, 16)

        @block.gpsimd
        def _(gpsimd):
            gpsimd.dma_start(wp[:, 0:wc], wp_d.ap()[:, 0:wc]).then_inc(dmaW, 16)
            gpsimd.dma_start(wp[:, wc:2 * wc],
                             wp_d.ap()[:, wc:2 * wc]).then_inc(dmaW, 16)
            for j in range(4):
                gpsimd.wait_ge(sQ, j // 2 + 1)
                sl = slice(j * 32, (j + 1) * 32)
                nc.gpsimd.tensor_tensor(d33[:, :, sl], d23[:, :, sl],
                                        a3[:, :, sl],
                                        op=ALU.mult).then_inc(sD, 1)

        @block.scalar
        def _(scalar):
            scalar.dma_start(misc[:, BS:MC],
                             misc_d.ap()[:, BS:MC]).then_inc(dmaM2, 16)
            # dummy ACT touching only DVE-memset data: pulls the
            # exp_and_others table load to t~0, hidden under the DMAs
            scalar.wait_ge(sOnes, 1)
            nc.scalar.activation(dummy[:], ones[0:1, 0:8], AF.Exp)
            scalar.wait_ge(dmaM, 16)
            nc.scalar.activation(xn[:, qs(0)], xt[:, qs(0)],
                                 AF.Tanh).then_inc(sA, 1)
            scalar.wait_ge(dmaM2, 16)
            nc.scalar.activation(xn[:, qs(1)], xt[:, qs(1)],
                                 AF.Tanh).then_inc(sA, 1)
            for h in range(NCH):
                scalar.wait_ge(sV, h + 1)
                nc.scalar.activation(d23[:, :, qs(h)], v3[:, :, qs(h)],
                                     AF.Square)
                nc.scalar.activation(a3[:, :, qs(h)], v3[:, :, qs(h)],
                                     AF.Abs).then_inc(sQ, 1)
            for j in range(4):
                scalar.wait_ge(sD, j + 1)
                sl = slice(j * 32, (j + 1) * 32)
                nc.scalar.activation(E3[:, :, sl], d33[:, :, sl],
                                     AF.Exp, scale=-1.0).then_inc(sE, 1)
            scalar.wait_ge(sP0, 1)
            nc.scalar.copy(outsb[:], psum0[:]).then_inc(sC, 1)

        @block.vector
        def _(vector):
            nc.vector.memset(ones[:], 1.0).then_inc(sOnes, 1)
            nc.vector.memset(wsrc[:], 0.5).then_inc(sOnes, 1)
            for h in range(NCH):
                vector.wait_ge(sA, h + 1)
                for g in range(G):
                    ins = nc.vector.tensor_scalar(
                        v[:, g * FQ + h * BS: g * FQ + (h + 1) * BS],
                        xn[:, qs(h)], float(grid_vals[g]), None,
                        op0=ALU.subtract)
                    if g == G - 1:
                        ins.then_inc(sV, 1)
            for h in range(NCH):
                for k in range(2):
                    j = h * 2 + k
                    vector.wait_ge(sE, j + 1)
                    sl = slice(j * 32, (j + 1) * 32)
                    # S = sum_g E; S >= 1.5 so fp32(S+1e-8) == S: skip eps
                    nc.vector.tensor_reduce(S[:, sl], E4[:, sl, :],
                                            axis=mybir.AxisListType.X,
                                            op=ALU.add)
                nc.vector.reciprocal_approx_fast(r[:, qs(h)], S[:, qs(h)])
                nc.vector.tensor_tensor(
                    En3[:, :, qs(h)], E3[:, :, qs(h)],
                    r[:, qs(h)].unsqueeze(1).broadcast_to((128, G, BS)),
                    op=ALU.mult).then_inc(sN, 1)

        @block.tensor
        def _(tensor):
            # bias first (only needs ones + misc), then both halves
            # accumulate into one psum bank; PE executes strictly in order
            tensor.wait_ge(dmaM2, 16)
            tensor.wait_ge(sOnes, 1)
            nc.tensor.matmul(psum0[:], ones[:], bias_row,
                             start=True, stop=False)
            # HAM warm-up: junk matmuls on a scratch bank while the
            # elementwise chain runs, so the real burst runs at 2.4 GHz
            tensor.wait_ge(sOnes, 2)
            for _ in range(int(os.environ.get('NKERN_WARM', '0'))):
                nc.tensor.matmul(scr_ps[:], wsrc[:, 0:BS], wsrc[:],
                                 start=True, stop=True)
            tensor.wait_ge(dmaW, 16)
            tensor.wait_ge(sN, 1)
            for g in range(G):
                nc.tensor.matmul(psum0[:],
                                 En[:, g * FQ: g * FQ + BS],
                                 wp[:, g * O: (g + 1) * O],
                                 start=False, stop=False)
            tensor.wait_ge(dmaW, 32)
            tensor.wait_ge(sN, 2)
            for g in range(G):
                ins = nc.tensor.matmul(psum0[:],
                                       En[:, g * FQ + BS: g * FQ + 2 * BS],
                                       wp[:, (G + g) * O: (G + g + 1) * O],
                                       start=False, stop=(g == G - 1))
            ins.then_inc(sP0, 1)

    nc.compile()
    return nc



def _build_program_rg(grid_vals):
    """RG layout: partitions p = (i_lo, g) with i_lo = i % 16, so the
    basis g-normalization sum becomes a PE matmul against a 0/1 mask
    (contract partitions, broadcast back over g) instead of a DVE
    strided reduce.  Free dim f = (c, b), i = c*16 + i_lo.

      SYNC:   x-half0 + aux(mask|grid|bias) DMA, out DMA
      SCALAR: x-half1 DMA, table dummy, tanh, Abs, Exp, psum copy
      DVE:    ones memset, v = xn - grid_p, v*v, 1/S (PSUM), normalize
      GPSIMD: weight DMAs, d3 = d2*|v|
      PE:     S = mask.T @ E per half, bias matmul, 16 bf16 matmuls
    """
    from contextlib import ExitStack

    from concourse import bacc, mybir

    f32 = mybir.dt.float32
    bf16 = mybir.dt.bfloat16
    AF = mybir.ActivationFunctionType
    ALU = mybir.AluOpType

    nc = bacc.Bacc("TRN2", target_bir_lowering=False, debug=False,
                   num_devices=N_CORES)

    IL, C = 16, I // 16          # i_lo count, chunk count
    FR = C * BS                  # 1024 free (c, b)
    HB = FR // 2                 # half size: 512
    XA, MA, GA, BA = 0, FR, FR + 128, FR + 129   # big_in col offsets
    BC = FR + 129 + O            # total cols: 1281
    big_d = nc.dram_tensor("big", [128, BC], f32, kind="ExternalInput")
    wr_d = nc.dram_tensor("wr", [128, C * O], bf16, kind="ExternalInput")
    out_d = nc.dram_tensor("out", [BS, O], f32, kind="ExternalOutput")
    wc = C * O // 2

    with ExitStack() as ctx:
        e = ctx.enter_context
        big = e(nc.sbuf_tensor([128, BC], f32))
        wr = e(nc.sbuf_tensor([128, C * O], bf16))
        ones = e(nc.sbuf_tensor([1, BS], f32))
        dummy = e(nc.sbuf_tensor([1, 8], f32))
        xn = e(nc.sbuf_tensor([128, FR], f32))
        v = e(nc.sbuf_tensor([128, FR], f32))
        d2 = e(nc.sbuf_tensor([128, FR], f32))
        av = e(nc.sbuf_tensor([128, FR], f32))
        d3 = e(nc.sbuf_tensor([128, FR], f32))
        E = e(nc.sbuf_tensor([128, FR], f32))
        r = e(nc.sbuf_tensor([128, FR], f32))
        En = e(nc.sbuf_tensor([128, FR], bf16))
        outsb = e(nc.sbuf_tensor([BS, O], f32))
        S_ps = e(nc.psum_tensor([128, FR], f32))
        out_ps = e(nc.psum_tensor([BS, O], f32))

        dmaX0 = e(nc.semaphore("dmaX0"))
        dmaX1 = e(nc.semaphore("dmaX1"))
        dmaA = e(nc.semaphore("dmaA"))
        dmaW = e(nc.semaphore("dmaW"))
        dmaO = e(nc.semaphore("dmaO"))
        sOnes = e(nc.semaphore("sOnes"))
        sA = e(nc.semaphore("sA"))
        sV = e(nc.semaphore("sV"))
        sQ = e(nc.semaphore("sQ"))
        sB = e(nc.semaphore("sB"))
        sD = e(nc.semaphore("sD"))
        sE = e(nc.semaphore("sE"))
        sS = e(nc.semaphore("sS"))
        sN = e(nc.semaphore("sN"))
        sP = e(nc.semaphore("sP"))
        sC = e(nc.semaphore("sC"))

        block = e(nc.Block(no_gpsimd_drain=True))

        mask_ap = big[:, MA:MA + 128]
        gv_ap = big[:, GA:GA + 1]
        bias_row = big[0:1, BA:BA + O]

        def hs(h):
            return slice(h * HB, (h + 1) * HB)

        @block.sync
        def _(sync):
            sync.dma_start(big[:, 0:HB], big_d.ap()[:, 0:HB]).then_inc(dmaX0, 16)
            sync.dma_start(big[:, MA:BC], big_d.ap()[:, MA:BC]).then_inc(dmaA, 16)
            sync.wait_ge(sC, 1)
            sync.dma_start(out_d.ap(), outsb[:]).then_inc(dmaO, 16)
            sync.wait_ge(dmaO, 16)

        @block.scalar
        def _(scalar):
            scalar.dma_start(big[:, HB:FR],
                             big_d.ap()[:, HB:FR]).then_inc(dmaX1, 16)
            scalar.wait_ge(sOnes, 1)
            nc.scalar.activation(dummy[:], ones[0:1, 0:8], AF.Exp)
            scalar.wait_ge(dmaX0, 16)
            nc.scalar.activation(xn[:, hs(0)], big[:, hs(0)],
                                 AF.Tanh).then_inc(sA, 1)
            scalar.wait_ge(dmaX1, 16)
            nc.scalar.activation(xn[:, hs(1)], big[:, hs(1)],
                                 AF.Tanh).then_inc(sA, 1)
            for h in range(2):
                scalar.wait_ge(sV, h + 1)
                nc.scalar.activation(av[:, hs(h)], v[:, hs(h)],
                                     AF.Abs).then_inc(sB, 1)
            for h in range(2):
                scalar.wait_ge(sD, h + 1)
                nc.scalar.activation(E[:, hs(h)], d3[:, hs(h)],
                                     AF.Exp, scale=-1.0).then_inc(sE, 1)
            scalar.wait_ge(sP, 1)
            nc.scalar.copy(outsb[:], out_ps[:]).then_inc(sC, 1)

        @block.vector
        def _(vector):
            nc.vector.memset(ones[:], 1.0).then_inc(sOnes, 1)
            nc.vector.memset(wsrc[:], 0.5).then_inc(sOnes, 1)
            vector.wait_ge(dmaA, 16)
            vector.wait_ge(sA, 1)
            nc.vector.tensor_scalar(v[:, hs(0)], xn[:, hs(0)], gv_ap, None,
                                    op0=ALU.subtract).then_inc(sV, 1)
            nc.vector.tensor_tensor(d2[:, hs(0)], v[:, hs(0)], v[:, hs(0)],
                                    op=ALU.mult).then_inc(sQ, 1)
            vector.wait_ge(sA, 2)
            nc.vector.tensor_scalar(v[:, hs(1)], xn[:, hs(1)], gv_ap, None,
                                    op0=ALU.subtract).then_inc(sV, 1)
            nc.vector.tensor_tensor(d2[:, hs(1)], v[:, hs(1)], v[:, hs(1)],
                                    op=ALU.mult).then_inc(sQ, 1)
            for h in range(2):
                vector.wait_ge(sS, h + 1)
                # S >= 1.5 here so fp32(S + 1e-8) == S: reference eps no-op
                nc.vector.reciprocal_approx_fast(r[:, hs(h)], S_ps[:, hs(h)])
                nc.vector.tensor_tensor(En[:, hs(h)], E[:, hs(h)],
                                        r[:, hs(h)],
                                        op=ALU.mult).then_inc(sN, 1)

        @block.gpsimd
        def _(gpsimd):
            gpsimd.dma_start(wr[:, 0:wc], wr_d.ap()[:, 0:wc]).then_inc(dmaW, 16)
            gpsimd.dma_start(wr[:, wc:2 * wc],
                             wr_d.ap()[:, wc:2 * wc]).then_inc(dmaW, 16)
            for h in range(2):
                gpsimd.wait_ge(sQ, h + 1)
                gpsimd.wait_ge(sB, h + 1)
                nc.gpsimd.tensor_tensor(d3[:, hs(h)], d2[:, hs(h)],
                                        av[:, hs(h)],
                                        op=ALU.mult).then_inc(sD, 1)

        @block.tensor
        def _(tensor):
            tensor.wait_ge(dmaA, 16)
            tensor.wait_ge(sE, 1)
            nc.tensor.matmul(S_ps[:, hs(0)], mask_ap, E[:, hs(0)],
                             start=True, stop=True).then_inc(sS, 1)
            tensor.wait_ge(sOnes, 1)
            nc.tensor.matmul(out_ps[:], ones[:], bias_row,
                             start=True, stop=False)
            tensor.wait_ge(sE, 2)
            nc.tensor.matmul(S_ps[:, hs(1)], mask_ap, E[:, hs(1)],
                             start=True, stop=True).then_inc(sS, 1)
            tensor.wait_ge(dmaW, 16)
            tensor.wait_ge(sN, 1)
            for c in range(C // 2):
                nc.tensor.matmul(out_ps[:], En[:, c * BS:(c + 1) * BS],
                                 wr[:, c * O:(c + 1) * O],
                                 start=False, stop=False)
            tensor.wait_ge(dmaW, 32)
            tensor.wait_ge(sN, 2)
            for c in range(C // 2, C):
                ins = nc.tensor.matmul(out_ps[:], En[:, c * BS:(c + 1) * BS],
                                       wr[:, c * O:(c + 1) * O],
                                       start=False, stop=(c == C - 1))
            ins.then_inc(sP, 1)

    nc.compile()
    return nc


def _pack_inputs_rg(x, spline_weight, spline_scaler, bias, grid_points):
    import ml_dtypes

    IL, C = 16, I // 16
    FR = C * BS
    BC = FR + 129 + O
    s_row = spline_scaler[0].astype(np.float32)
    xdiv_all = (x.astype(np.float32) / s_row[None, :])
    mask = np.kron(np.eye(IL, dtype=np.float32),
                   np.ones((G, G), dtype=np.float32))
    gridvec = np.tile(grid_points.astype(np.float32), IL)
    wr = spline_weight.astype(np.float32).transpose(1, 2, 0)     # (I, G, O)
    wr = wr.reshape(C, IL, G, O).transpose(1, 2, 0, 3)           # (IL,G,C,O)
    wr = np.ascontiguousarray(wr.reshape(128, C * O)).astype(
        ml_dtypes.bfloat16)

    in_maps = []
    for cr in range(N_CORES):
        xd = xdiv_all[cr * BS:(cr + 1) * BS]                     # (BS, I)
        xr = xd.T.reshape(C, IL, BS)                             # (C,IL,BS)
        xr = np.broadcast_to(xr.transpose(1, 0, 2)[:, None, :, :],
                             (IL, G, C, BS))                     # (IL,G,C,BS)
        big = np.zeros((128, BC), dtype=np.float32)
        big[:, 0:FR] = xr.reshape(128, FR)
        big[:, FR:FR + 128] = mask
        big[:, FR + 128] = gridvec
        big[0, FR + 129:FR + 129 + O] = bias.astype(np.float32)
        in_maps.append({"big": big, "wr": wr})
    return in_maps


def _pack_inputs(x, spline_weight, spline_scaler, bias, grid_points,
                 mm_bf16=True):
    import ml_dtypes

    grid_f = grid_points.astype(np.float32)

    FQ = NCH * BS
    MC = FQ + O + G
    s_row = spline_scaler[0].astype(np.float32)                  # (I,)
    xs_all = (x.astype(np.float32) / s_row[None, :])             # host divide
    wp = spline_weight.astype(np.float32).transpose(1, 2, 0)     # (I, G, O)
    wp = wp.reshape(NCH, 128, G, O).transpose(1, 0, 2, 3)        # (128,NCH,G,O)
    wp = np.ascontiguousarray(wp.reshape(128, NCH * G * O))
    if mm_bf16:
        wp = wp.astype(ml_dtypes.bfloat16)

    in_maps = []
    for c in range(N_CORES):
        xs = xs_all[c * BS:(c + 1) * BS]                         # (BS, I)
        xt = xs.T.reshape(NCH, 128, BS).transpose(1, 0, 2)       # (128,NCH,BS)
        misc = np.zeros((128, MC), dtype=np.float32)
        misc[:, 0:FQ] = xt.reshape(128, FQ)
        misc[0, FQ:FQ + O] = bias.astype(np.float32)
        misc[:, FQ + O:FQ + O + G] = grid_f[None, :]
        in_maps.append({"misc": misc, "wp": wp})
    return in_maps


LAST_RESULTS = None


def kernel(x, spline_weight, spline_scaler, bias, grid_points):
    global LAST_RESULTS
    x = np.asarray(x, dtype=np.float32)
    spline_weight = np.asarray(spline_weight, dtype=np.float32)
    spline_scaler = np.asarray(spline_scaler, dtype=np.float32)
    bias = np.asarray(bias, dtype=np.float32)
    grid_points = np.asarray(grid_points, dtype=np.float32)

    if (x.shape != (B, I) or spline_weight.shape != (O, I, G)
            or not np.array_equal(spline_scaler,
                                  np.broadcast_to(spline_scaler[0:1, :],
                                                  spline_scaler.shape))):
        return _reference_numpy(x, spline_weight, spline_scaler, bias,
                                grid_points)

    from concourse.bass_utils import run_bass_kernel_spmd

    impl = os.environ.get("NKERN_IMPL", "raw")
    mm_bf16 = os.environ.get("NKERN_PREC", "bf16") != "fp32"
    key = (impl, mm_bf16, grid_points.tobytes())
    if impl == "rg":
        if key not in _CACHE:
            _CACHE[key] = _build_program_rg([float(v) for v in grid_points])
        in_maps = _pack_inputs_rg(x, spline_weight, spline_scaler, bias,
                                  grid_points)
    else:
        if key not in _CACHE:
            _CACHE[key] = _build_program_raw([float(v) for v in grid_points],
                                             mm_bf16=mm_bf16)
        in_maps = _pack_inputs(x, spline_weight, spline_scaler, bias,
                               grid_points, mm_bf16=mm_bf16)
    nc = _CACHE[key]

    trace = bool(int(os.environ.get("NKERN_TRACE", "0")))
    if trace:
        _ensure_axon_ntff_hook()
    res = run_bass_kernel_spmd(nc, in_maps, list(range(N_CORES)), trace=trace)
    LAST_RESULTS = res
    return np.concatenate([res.results[c]["out"] for c in range(N_CORES)],
                          axis=0)



# revision 7
# speedup vs baseline: 1.4600x; 1.4600x over previous
"""Trainium2 Bass kernel for EnhancedKANLayer (spline-order-3 KAN layer).

Reference computation (fp32):
    x_norm = tanh(x[:, None, :] / scaler[None, :, :])          # (B, O, I)
    d      = |x_norm[..., None] - grid|                        # (B, O, I, G)
    b      = exp(-d**3);  bhat = b / (sum_g b + 1e-8)
    out    = einsum('boig,oig->bo', bhat, W) + bias

With scaler uniform across O (as produced by setup_inputs), x_norm is
O-independent.  The G=8 normalized basis functions bhat_g(t) are fixed
smooth scalar functions of t = tanh(x) on (-1, 1), so we replace them by
a degree-D polynomial fit (Chebyshev fit, converted to monomial basis;
coefficients are small so monomials are bf16-safe):

    bhat_g(t) ~= sum_k c[k,g] t^k
    out[b,o]  = sum_{i,k} t_{bi}^k A[o,i,k] + bias_eff[o]
    A[o,i,k]  = sum_g c[k,g] W[o,i,g],  bias_eff = bias + sum_i A[:,i,0]

This kills the whole elementwise basis pipeline (sub/abs/square/mult/
exp/reduce/recip/normalize over B*I*G elements) and leaves: one tanh,
a handful of bf16 power products over B*I elements, and K'*2 small
accumulating matmuls.  Fit error at deg 6 gives end-to-end rel ~3.6e-3
(measured against the jax reference; bf16 matmul floor is ~2.4e-3).

Sharding: data-parallel over batch across 8 NeuronCores (64 rows/core,
A replicated).  Per core, raw-bacc program (manual semaphores):
  SYNC:   x DMA (two column-halves = the two i-chunks), out DMA
  SCALAR: weight half-a DMA (k=1,2), ACT table prefetch dummy, tanh
  DVE:    ones memset, x2/x3/x4/P6 bf16 products, psum->sbuf copy
  GPSIMD: weight half-b DMA (k>=3 + bias rows), P5 products
  PE:     13 accumulating bf16 matmuls (K'=6 powers x 2 i-chunks +
          one 2-row Kahan-split bias matmul vs a ones vector)
Bias is applied exactly via the two-row bf16 Kahan split (hi+lo).
Falls back to a pure-numpy reference path if scaler is not uniform
across O or shapes differ (never hit by the real input distribution).
"""

import os
import sys
import types

import numpy as np

N_CORES = 8
B, I, O, G = 512, 256, 128, 8
BS = B // N_CORES          # batch rows per core (64)
NCH = I // 128             # i-chunks of 128 partitions (2)
EPS = 1e-8

_CACHE = {}
_FIT_CACHE = {}


def _ensure_axon_ntff_hook():
    """Register the NTFF profiling hook (missing antenv.axon_hooks shim).
    Only needed for traced runs; harmless otherwise."""
    try:
        import antenv
        if 'antenv.axon_hooks' not in sys.modules:
            mod = types.ModuleType('antenv.axon_hooks')
            holder = [None]
            mod.set_axon_ntff_profile_hook = lambda h: holder.__setitem__(0, h)
            mod.get_axon_ntff_profile_hook = lambda: holder[0]
            sys.modules['antenv.axon_hooks'] = mod
            antenv.axon_hooks = mod
        mod = sys.modules['antenv.axon_hooks']
        if mod.get_axon_ntff_profile_hook() is None:
            from trn_agent_boot.trn_boot import _ntff_profile_via_ctypes
            so = '/opt/axon/libaxon_pjrt.so'
            if os.path.exists(so):
                mod.set_axon_ntff_profile_hook(_ntff_profile_via_ctypes(so))
    except Exception:
        pass


def _reference_numpy(x, spline_weight, spline_scaler, bias, grid_points):
    """General fallback, mirrors the jax reference in numpy (fp32)."""
    x = x.astype(np.float32)
    xn = np.tanh(x[:, None, :] / spline_scaler[None, :, :])          # (B,O,I)
    d = np.abs(xn[..., None] - grid_points)                           # (B,O,I,G)
    b = np.exp(-(d ** 3))
    bhat = b / (b.sum(axis=-1, keepdims=True) + EPS)
    out = np.einsum('boig,oig->bo', bhat, spline_weight, optimize=True)
    return (out + bias[None, :]).astype(np.float32)


def _fit_mono(grid_points, deg):
    """Chebyshev-fit the G normalized basis functions on t in [-1,1],
    return monomial coefficients mono[k, g] (k = 0..deg)."""
    key = (grid_points.tobytes(), deg)
    if key in _FIT_CACHE:
        return _FIT_CACHE[key]
    import numpy.polynomial.chebyshev as C
    g = grid_points.astype(np.float64)
    ts = np.cos(np.pi * (np.arange(4000) + 0.5) / 4000)
    d = np.abs(ts[:, None] - g[None, :])
    b = np.exp(-(d ** 3))
    bh = b / (b.sum(-1, keepdims=True) + EPS)
    mono = np.stack(
        [C.cheb2poly(C.chebfit(ts, bh[:, j], deg)) for j in range(len(g))],
        axis=1)                                                  # (deg+1, G)
    _FIT_CACHE[key] = mono
    return mono


def _build_program(deg, waitout):
    """Raw bacc program for the polynomial-KAN kernel; deg+1 = K powers.

    Power products (all bf16, halves h = i-chunk):
      T = tanh(x)          [ACT]
      x2 = T*T, x3 = x2*T, x4 = x2*x2, P6 = x3*x3   [DVE]
      P5 = x2*x3           [GPSIMD]
      (deg 7 adds P7 = x3*x4 on GPSIMD; deg 5 drops P6)
    """
    from contextlib import ExitStack

    from concourse import bacc, mybir

    f32 = mybir.dt.float32
    bf16 = mybir.dt.bfloat16
    AF = mybir.ActivationFunctionType
    ALU = mybir.AluOpType

    KP = deg                     # number of non-constant powers (k = 1..KP)
    assert 4 <= KP <= 7
    WA = 2 * 2 * 128             # k=1,2 cols (both chunks) -> Sync queue
    WB = WA + (2 if KP >= 4 else 1) * 2 * 128   # k=3,4 -> Scalar queue
    WCOLS = KP * 2 * 128 + 128   # + bias block (2 Kahan rows x 128 o)

    nc = bacc.Bacc("TRN2", target_bir_lowering=False, debug=False,
                   num_devices=N_CORES)

    x_d = nc.dram_tensor("x", [128, 128], f32, kind="ExternalInput")
    wr_d = nc.dram_tensor("wr", [128, WCOLS], bf16, kind="ExternalInput")
    out_d = nc.dram_tensor("out", [BS, O], f32, kind="ExternalOutput")

    def wcol(k, ch):
        return ((k - 1) * 2 + ch) * 128

    bias_col = KP * 2 * 128

    with ExitStack() as ctx:
        e = ctx.enter_context
        xs = e(nc.sbuf_tensor([128, 128], f32))
        T = e(nc.sbuf_tensor([128, 128], bf16))
        x2 = e(nc.sbuf_tensor([128, 128], bf16))
        x3 = e(nc.sbuf_tensor([128, 128], bf16))
        x4 = e(nc.sbuf_tensor([128, 128], bf16))
        P5 = e(nc.sbuf_tensor([128, 128], bf16))
        P6 = e(nc.sbuf_tensor([128, 128], bf16))
        P7 = e(nc.sbuf_tensor([128, 128], bf16))
        wr = e(nc.sbuf_tensor([128, WCOLS], bf16))
        ones = e(nc.sbuf_tensor([2, BS], bf16))
        dummy = e(nc.sbuf_tensor([1, 8], f32))
        outsb = e(nc.sbuf_tensor([BS, O], f32))
        psum = e(nc.psum_tensor([BS, O], f32))

        dmaX0 = e(nc.semaphore("dmaX0"))
        dmaX1 = e(nc.semaphore("dmaX1"))
        dmaWa = e(nc.semaphore("dmaWa"))
        dmaWb = e(nc.semaphore("dmaWb"))
        dmaWc = e(nc.semaphore("dmaWc"))
        dmaO = e(nc.semaphore("dmaO"))
        sOnes = e(nc.semaphore("sOnes"))
        sT = e(nc.semaphore("sT"))
        s2 = e(nc.semaphore("s2"))
        s3 = e(nc.semaphore("s3"))
        s4 = e(nc.semaphore("s4"))
        s5 = e(nc.semaphore("s5"))
        s6 = e(nc.semaphore("s6"))
        s7 = e(nc.semaphore("s7"))
        sP = e(nc.semaphore("sP"))
        sC = e(nc.semaphore("sC"))

        block = e(nc.Block(no_gpsimd_drain=True))

        def hs(h):
            return slice(h * BS, (h + 1) * BS)

        dmaX = (dmaX0, dmaX1)

        @block.sync
        def _(sync):
            # x first so tanh isn't starved by weight-stream competition
            sync.dma_start(xs[:, hs(0)], x_d.ap()[:, hs(0)]).then_inc(dmaX0, 16)
            sync.dma_start(wr[:, 0:WA], wr_d.ap()[:, 0:WA]).then_inc(dmaWa, 16)
            sync.wait_ge(sC, 1)
            sync.dma_start(out_d.ap(), outsb[:]).then_inc(dmaO, 16)
            if waitout:
                sync.wait_ge(dmaO, 16)

        @block.scalar
        def _(scalar):
            scalar.dma_start(xs[:, hs(1)], x_d.ap()[:, hs(1)]).then_inc(dmaX1, 16)
            scalar.dma_start(wr[:, WA:WB], wr_d.ap()[:, WA:WB]).then_inc(dmaWb, 16)
            # dummy ACT: guarantees the tanh table set is resident before x
            # lands (the ACT_TABLE_LOAD pseudo-inst hoists to stream start)
            scalar.wait_ge(sOnes, 1)
            nc.scalar.activation(dummy[:], ones[0:1, 0:8], AF.Tanh)
            for h in range(2):
                scalar.wait_ge(dmaX[h], 16)
                nc.scalar.activation(T[:, hs(h)], xs[:, hs(h)],
                                     AF.Tanh).then_inc(sT, 1)

        @block.vector
        def _(vector):
            nc.vector.memset(ones[:], 1.0).then_inc(sOnes, 1)
            for h in range(2):
                vector.wait_ge(sT, h + 1)
                nc.vector.tensor_tensor(x2[:, hs(h)], T[:, hs(h)], T[:, hs(h)],
                                        op=ALU.mult).then_inc(s2, 1)
                nc.vector.tensor_tensor(x3[:, hs(h)], x2[:, hs(h)], T[:, hs(h)],
                                        op=ALU.mult).then_inc(s3, 1)
                if KP >= 4:
                    nc.vector.tensor_tensor(x4[:, hs(h)], x2[:, hs(h)],
                                            x2[:, hs(h)],
                                            op=ALU.mult).then_inc(s4, 1)
                if KP >= 6:
                    nc.vector.tensor_tensor(P6[:, hs(h)], x3[:, hs(h)],
                                            x3[:, hs(h)],
                                            op=ALU.mult).then_inc(s6, 1)
            vector.wait_ge(sP, 1)
            nc.vector.tensor_scalar(outsb[:], psum[:], 1.0, None,
                                    op0=ALU.mult).then_inc(sC, 1)

        @block.gpsimd
        def _(gpsimd):
            gpsimd.dma_start(wr[:, WB:WCOLS],
                             wr_d.ap()[:, WB:WCOLS]).then_inc(dmaWc, 16)
            for h in range(2):
                if KP >= 5:
                    gpsimd.wait_ge(s3, h + 1)
                    nc.gpsimd.tensor_tensor(P5[:, hs(h)], x2[:, hs(h)],
                                            x3[:, hs(h)],
                                            op=ALU.mult).then_inc(s5, 1)
                if KP >= 7:
                    gpsimd.wait_ge(s4, h + 1)
                    nc.gpsimd.tensor_tensor(P7[:, hs(h)], x3[:, hs(h)],
                                            x4[:, hs(h)],
                                            op=ALU.mult).then_inc(s7, 1)

        pw = {1: T, 2: x2, 3: x3, 4: x4, 5: P5, 6: P6, 7: P7}
        psem = {1: sT, 2: s2, 3: s3, 4: s4, 5: s5, 6: s6, 7: s7}

        @block.tensor
        def _(tensor):
            # three weight groups, each consumed in chunk-then-k order as
            # its DMA lands; k=0 denotes the 2-row Kahan bias matmul
            grp_a = [(1, 0), (1, 1), (2, 0), (2, 1)]
            grp_b = [(k, ch) for k in range(3, min(KP, 4) + 1)
                     for ch in range(2)]
            grp_c = ([(0, 0)] + [(k, ch) for k in range(5, KP + 1)
                                 for ch in range(2)])
            waited = {}

            def emit(k, ch, start, stop):
                if k == 0:
                    tensor.wait_ge(sOnes, 1)
                    return nc.tensor.matmul(
                        psum[:], ones[:], wr[0:2, bias_col:bias_col + 128],
                        start=start, stop=stop)
                need = ch + 1
                if waited.get(k, 0) < need:
                    tensor.wait_ge(psem[k], need)
                    waited[k] = need
                return nc.tensor.matmul(
                    psum[:], pw[k][:, hs(ch)],
                    wr[:, wcol(k, ch):wcol(k, ch) + 128],
                    start=start, stop=stop)

            sched = [(dmaWa, grp_a), (dmaWb, grp_b), (dmaWc, grp_c)]
            n_total = sum(len(g) for _, g in sched)
            j = 0
            for sem, grp in sched:
                if grp:
                    tensor.wait_ge(sem, 16)
                for (k, ch) in grp:
                    ins = emit(k, ch, j == 0, j == n_total - 1)
                    j += 1
            ins.then_inc(sP, 1)

    nc.compile()
    return nc


def _pack_inputs(x, spline_weight, spline_scaler, bias, grid_points, deg):
    import ml_dtypes

    KP = deg
    mono = _fit_mono(grid_points.astype(np.float32), deg)        # (deg+1, G)
    A = np.einsum('kg,oig->oik', mono.astype(np.float64),
                  spline_weight.astype(np.float64))              # (O, I, K+1)
    bias_eff = (bias.astype(np.float64) + A[:, :, 0].sum(axis=1))

    WCOLS = KP * 2 * 128 + 128
    wr = np.zeros((128, WCOLS), dtype=np.float32)
    for k in range(1, KP + 1):
        for ch in range(NCH):
            # stationary for (k, ch): [i_in_chunk, o]
            base = ((k - 1) * 2 + ch) * 128
            wr[:, base:base + 128] = A[:, ch * 128:(ch + 1) * 128, k].T
    bcol = KP * 2 * 128
    bhi = bias_eff.astype(np.float32).astype(ml_dtypes.bfloat16)
    blo = (bias_eff - bhi.astype(np.float64)).astype(np.float32)
    wr[0, bcol:bcol + 128] = bhi.astype(np.float32)
    wr[1, bcol:bcol + 128] = blo
    wrb = wr.astype(ml_dtypes.bfloat16)

    s_row = spline_scaler[0].astype(np.float32)                  # (I,)
    xs_all = (x.astype(np.float32) / s_row[None, :])             # host divide
    in_maps = []
    for c in range(N_CORES):
        xd = xs_all[c * BS:(c + 1) * BS]                         # (BS, I)
        xt = xd.T.reshape(NCH, 128, BS).transpose(1, 0, 2)       # (128,NCH,BS)
        in_maps.append({"x": np.ascontiguousarray(xt.reshape(128, 128)),
                        "wr": wrb})
    return in_maps


LAST_RESULTS = None


def kernel(x, spline_weight, spline_scaler, bias, grid_points):
    global LAST_RESULTS
    x = np.asarray(x, dtype=np.float32)
    spline_weight = np.asarray(spline_weight, dtype=np.float32)
    spline_scaler = np.asarray(spline_scaler, dtype=np.float32)
    bias = np.asarray(bias, dtype=np.float32)
    grid_points = np.asarray(grid_points, dtype=np.float32)

    if (x.shape != (B, I) or spline_weight.shape != (O, I, G)
            or not np.array_equal(spline_scaler,
                                  np.broadcast_to(spline_scaler[0:1, :],
                                                  spline_scaler.shape))):
        return _reference_numpy(x, spline_weight, spline_scaler, bias,
                                grid_points)

    from concourse.bass_utils import run_bass_kernel_spmd

    deg = int(os.environ.get("NKERN_DEG", "6"))
    waitout = bool(int(os.environ.get("NKERN_WAITOUT", "1")))
    key = (deg, waitout)
    if key not in _CACHE:
        _CACHE[key] = _build_program(deg, waitout)
    nc = _CACHE[key]
    in_maps = _pack_inputs(x, spline_weight, spline_scaler, bias,
                           grid_points, deg)

    trace = bool(int(os.environ.get("NKERN_TRACE", "0")))
    if trace:
        _ensure_axon_ntff_hook()
    res = run_bass_kernel_spmd(nc, in_maps, list(range(N_CORES)), trace=trace)
    LAST_RESULTS = res
    return np.concatenate([res.results[c]["out"] for c in range(N_CORES)],
                          axis=0)


# revision 12
# speedup vs baseline: 1.5061x; 1.0315x over previous
"""Trainium2 Bass kernel for EnhancedKANLayer (spline-order-3 KAN layer).

Reference computation (fp32):
    x_norm = tanh(x[:, None, :] / scaler[None, :, :])          # (B, O, I)
    d      = |x_norm[..., None] - grid|                        # (B, O, I, G)
    b      = exp(-d**3);  bhat = b / (sum_g b + 1e-8)
    out    = einsum('boig,oig->bo', bhat, W) + bias

With scaler uniform across O (as produced by setup_inputs), x_norm is
O-independent.  The G=8 normalized basis functions bhat_g(t) are fixed
smooth scalar functions of t = tanh(x) on (-1, 1), so we replace them by
a degree-D polynomial fit (Chebyshev fit, converted to monomial basis;
coefficients are small so monomials are bf16-safe):

    bhat_g(t) ~= sum_k c[k,g] t^k
    out[b,o]  = sum_{i,k} t_{bi}^k A[o,i,k] + bias_eff[o]
    A[o,i,k]  = sum_g c[k,g] W[o,i,g],  bias_eff = bias + sum_i A[:,i,0]

This kills the whole elementwise basis pipeline (sub/abs/square/mult/
exp/reduce/recip/normalize over B*I*G elements) and leaves: one tanh,
a handful of bf16 power products over B*I elements, and K'*2 small
accumulating matmuls.  Fit error at deg 6 gives end-to-end rel ~3.6e-3
(measured against the jax reference; bf16 matmul floor is ~2.4e-3).

Sharding: data-parallel over batch across 8 NeuronCores (64 rows/core,
A replicated).  Per core, raw-bacc program (manual semaphores):
  SYNC:   x DMA (two column-halves = the two i-chunks), out DMA
  SCALAR: weight half-a DMA (k=1,2), ACT table prefetch dummy, tanh
  DVE:    ones memset, x2/x3/x4/P6 bf16 products, psum->sbuf copy
  GPSIMD: weight half-b DMA (k>=3 + bias rows), P5 products
  PE:     13 accumulating bf16 matmuls (K'=6 powers x 2 i-chunks +
          one 2-row Kahan-split bias matmul vs a ones vector)
Bias is applied exactly via the two-row bf16 Kahan split (hi+lo).
Falls back to a pure-numpy reference path if scaler is not uniform
across O or shapes differ (never hit by the real input distribution).
"""

import os
import sys
import types

import numpy as np

N_CORES = 8
B, I, O, G = 512, 256, 128, 8
BS = B // N_CORES          # batch rows per core (64)
NCH = I // 128             # i-chunks of 128 partitions (2)
EPS = 1e-8

_CACHE = {}
_FIT_CACHE = {}


def _ensure_axon_ntff_hook():
    """Register the NTFF profiling hook (missing antenv.axon_hooks shim).
    Only needed for traced runs; harmless otherwise."""
    try:
        import antenv
        if 'antenv.axon_hooks' not in sys.modules:
            mod = types.ModuleType('antenv.axon_hooks')
            holder = [None]
            mod.set_axon_ntff_profile_hook = lambda h: holder.__setitem__(0, h)
            mod.get_axon_ntff_profile_hook = lambda: holder[0]
            sys.modules['antenv.axon_hooks'] = mod
            antenv.axon_hooks = mod
        mod = sys.modules['antenv.axon_hooks']
        if mod.get_axon_ntff_profile_hook() is None:
            from trn_agent_boot.trn_boot import _ntff_profile_via_ctypes
            so = '/opt/axon/libaxon_pjrt.so'
            if os.path.exists(so):
                mod.set_axon_ntff_profile_hook(_ntff_profile_via_ctypes(so))
    except Exception:
        pass


def _reference_numpy(x, spline_weight, spline_scaler, bias, grid_points):
    """General fallback, mirrors the jax reference in numpy (fp32)."""
    x = x.astype(np.float32)
    xn = np.tanh(x[:, None, :] / spline_scaler[None, :, :])          # (B,O,I)
    d = np.abs(xn[..., None] - grid_points)                           # (B,O,I,G)
    b = np.exp(-(d ** 3))
    bhat = b / (b.sum(axis=-1, keepdims=True) + EPS)
    out = np.einsum('boig,oig->bo', bhat, spline_weight, optimize=True)
    return (out + bias[None, :]).astype(np.float32)


def _fit_mono(grid_points, deg):
    """Chebyshev-fit the G normalized basis functions on t in [-1,1],
    return monomial coefficients mono[k, g] (k = 0..deg)."""
    key = (grid_points.tobytes(), deg)
    if key in _FIT_CACHE:
        return _FIT_CACHE[key]
    import numpy.polynomial.chebyshev as C
    g = grid_points.astype(np.float64)
    ts = np.cos(np.pi * (np.arange(4000) + 0.5) / 4000)
    d = np.abs(ts[:, None] - g[None, :])
    b = np.exp(-(d ** 3))
    bh = b / (b.sum(-1, keepdims=True) + EPS)
    mono = np.stack(
        [C.cheb2poly(C.chebfit(ts, bh[:, j], deg)) for j in range(len(g))],
        axis=1)                                                  # (deg+1, G)
    _FIT_CACHE[key] = mono
    return mono


def _build_program(deg, waitout):
    """Raw bacc program for the polynomial-KAN kernel; deg+1 = K powers.

    Power products (all bf16, halves h = i-chunk):
      T = tanh(x)          [ACT]
      x2 = T*T, x3 = x2*T, x4 = x2*x2, P6 = x3*x3   [DVE]
      P5 = x2*x3           [GPSIMD]
      (deg 7 adds P7 = x3*x4 on GPSIMD; deg 5 drops P6)
    """
    from contextlib import ExitStack

    from concourse import bacc, mybir

    f32 = mybir.dt.float32
    f16 = mybir.dt.float16
    AF = mybir.ActivationFunctionType
    ALU = mybir.AluOpType

    KP = deg                     # number of non-constant powers (k = 1..KP)
    assert 4 <= KP <= 7
    WA = 2 * 2 * 128             # k=1,2 cols (both chunks) -> Sync queue
    WB = WA + (2 if KP >= 4 else 1) * 2 * 128   # k=3,4 -> Scalar queue
    WCOLS = KP * 2 * 128 + 128   # + bias block (2 Kahan rows x 128 o)

    nc = bacc.Bacc("TRN2", target_bir_lowering=False, debug=False,
                   num_devices=N_CORES)

    x_d = nc.dram_tensor("x", [128, 128], f16, kind="ExternalInput")
    wr_d = nc.dram_tensor("wr", [128, WCOLS], f16, kind="ExternalInput")
    out_d = nc.dram_tensor("out", [BS, O], f32, kind="ExternalOutput")

    def wcol(k, ch):
        return ((k - 1) * 2 + ch) * 128

    bias_col = KP * 2 * 128

    with ExitStack() as ctx:
        e = ctx.enter_context
        xs = e(nc.sbuf_tensor([128, 128], f16))
        T = e(nc.sbuf_tensor([128, 128], f16))
        x2 = e(nc.sbuf_tensor([128, 128], f16))
        x3 = e(nc.sbuf_tensor([128, 128], f16))
        x4 = e(nc.sbuf_tensor([128, 128], f16))
        P5 = e(nc.sbuf_tensor([128, 128], f16))
        P6 = e(nc.sbuf_tensor([128, 128], f16))
        P7 = e(nc.sbuf_tensor([128, 128], f16))
        wr = e(nc.sbuf_tensor([128, WCOLS], f16))
        ones = e(nc.sbuf_tensor([2, BS], f16))
        jnkt = e(nc.sbuf_tensor([2, 256], f16))
        scr = e(nc.psum_tensor([BS, 256], f32))
        dummy = e(nc.sbuf_tensor([1, 8], f32))
        outsb = e(nc.sbuf_tensor([BS, O], f32))
        psum = e(nc.psum_tensor([BS, O], f32))

        dmaX0 = e(nc.semaphore("dmaX0"))
        dmaX1 = e(nc.semaphore("dmaX1"))
        dmaWa = e(nc.semaphore("dmaWa"))
        dmaWb = e(nc.semaphore("dmaWb"))
        dmaWc = e(nc.semaphore("dmaWc"))
        dmaO = e(nc.semaphore("dmaO"))
        sOnes = e(nc.semaphore("sOnes"))
        sT = e(nc.semaphore("sT"))
        s2 = e(nc.semaphore("s2"))
        s3 = e(nc.semaphore("s3"))
        s4 = e(nc.semaphore("s4"))
        s5 = e(nc.semaphore("s5"))
        s6 = e(nc.semaphore("s6"))
        s7 = e(nc.semaphore("s7"))
        sP = e(nc.semaphore("sP"))
        sC = e(nc.semaphore("sC"))

        block = e(nc.Block(no_gpsimd_drain=True))

        def hs(h):
            return slice(h * BS, (h + 1) * BS)

        dmaX = (dmaX0, dmaX1)

        @block.sync
        def _(sync):
            # x first so tanh isn't starved by weight-stream competition
            sync.dma_start(xs[:, hs(0)], x_d.ap()[:, hs(0)]).then_inc(dmaX0, 16)
            sync.dma_start(wr[:, 0:WA], wr_d.ap()[:, 0:WA]).then_inc(dmaWa, 16)
            sync.wait_ge(sC, 1)
            sync.dma_start(out_d.ap(), outsb[:]).then_inc(dmaO, 16)
            if waitout:
                sync.wait_ge(dmaO, 16)

        @block.scalar
        def _(scalar):
            scalar.dma_start(xs[:, hs(1)], x_d.ap()[:, hs(1)]).then_inc(dmaX1, 16)
            scalar.dma_start(wr[:, WA:WB], wr_d.ap()[:, WA:WB]).then_inc(dmaWb, 16)
            # dummy ACT: guarantees the tanh table set is resident before x
            # lands (the ACT_TABLE_LOAD pseudo-inst hoists to stream start)
            scalar.wait_ge(sOnes, 1)
            nc.scalar.activation(dummy[:], ones[0:1, 0:8], AF.Tanh)
            for h in range(2):
                scalar.wait_ge(dmaX[h], 16)
                nc.scalar.activation(T[:, hs(h)], xs[:, hs(h)],
                                     AF.Tanh).then_inc(sT, 1)

        @block.vector
        def _(vector):
            nc.vector.memset(ones[:], 1.0).then_inc(sOnes, 1)
            nc.vector.memset(jnkt[:], 0.5).then_inc(sOnes, 1)
            for h in range(2):
                vector.wait_ge(sT, h + 1)
                nc.vector.tensor_tensor(x2[:, hs(h)], T[:, hs(h)], T[:, hs(h)],
                                        op=ALU.mult).then_inc(s2, 1)
                nc.vector.tensor_tensor(x3[:, hs(h)], x2[:, hs(h)], T[:, hs(h)],
                                        op=ALU.mult).then_inc(s3, 1)
                if KP >= 4:
                    nc.vector.tensor_tensor(x4[:, hs(h)], x2[:, hs(h)],
                                            x2[:, hs(h)],
                                            op=ALU.mult).then_inc(s4, 1)
                if KP >= 6:
                    nc.vector.tensor_tensor(P6[:, hs(h)], x3[:, hs(h)],
                                            x3[:, hs(h)],
                                            op=ALU.mult).then_inc(s6, 1)
            vector.wait_ge(sP, 1)
            nc.vector.tensor_scalar(outsb[:], psum[:], 1.0, None,
                                    op0=ALU.mult).then_inc(sC, 1)

        @block.gpsimd
        def _(gpsimd):
            gpsimd.dma_start(wr[:, WB:WCOLS],
                             wr_d.ap()[:, WB:WCOLS]).then_inc(dmaWc, 16)
            for h in range(2):
                if KP >= 5:
                    gpsimd.wait_ge(s3, h + 1)
                    nc.gpsimd.tensor_tensor(P5[:, hs(h)], x2[:, hs(h)],
                                            x3[:, hs(h)],
                                            op=ALU.mult).then_inc(s5, 1)
                if KP >= 7:
                    gpsimd.wait_ge(s4, h + 1)
                    nc.gpsimd.tensor_tensor(P7[:, hs(h)], x3[:, hs(h)],
                                            x4[:, hs(h)],
                                            op=ALU.mult).then_inc(s7, 1)

        pw = {1: T, 2: x2, 3: x3, 4: x4, 5: P5, 6: P6, 7: P7}
        psem = {1: sT, 2: s2, 3: s3, 4: s4, 5: s5, 6: s6, 7: s7}

        @block.tensor
        def _(tensor):
            # three weight groups, each consumed in chunk-then-k order as
            # its DMA lands; k=0 denotes the 2-row Kahan bias matmul
            grp_a = [(1, 0), (1, 1), (2, 0), (2, 1)]
            grp_b = [(k, ch) for k in range(3, min(KP, 4) + 1)
                     for ch in range(2)]
            grp_c = ([(0, 0)] + [(k, ch) for k in range(5, KP + 1)
                                 for ch in range(2)])
            waited = {}

            def emit(k, ch, start, stop):
                if k == 0:
                    tensor.wait_ge(sOnes, 1)
                    return nc.tensor.matmul(
                        psum[:], ones[:], wr[0:2, bias_col:bias_col + 128],
                        start=start, stop=stop)
                need = ch + 1
                if waited.get(k, 0) < need:
                    tensor.wait_ge(psem[k], need)
                    waited[k] = need
                return nc.tensor.matmul(
                    psum[:], pw[k][:, hs(ch)],
                    wr[:, wcol(k, ch):wcol(k, ch) + 128],
                    start=start, stop=stop)

            # p-state warmup: junk matmuls on a scratch bank keep PE busy
            # through the weight-DMA wait so the real burst runs hot
            n_warm = int(os.environ.get("NKERN_WARM", "10"))
            if n_warm:
                tensor.wait_ge(sOnes, 2)
                for _ in range(n_warm):
                    nc.tensor.matmul(scr[:], ones[:], jnkt[:],
                                     start=True, stop=True)

            sched = [(dmaWa, grp_a), (dmaWb, grp_b), (dmaWc, grp_c)]
            n_total = sum(len(g) for _, g in sched)
            j = 0
            for sem, grp in sched:
                if grp:
                    tensor.wait_ge(sem, 16)
                for (k, ch) in grp:
                    ins = emit(k, ch, j == 0, j == n_total - 1)
                    j += 1
            ins.then_inc(sP, 1)

    nc.compile()
    return nc


def _pack_inputs(x, spline_weight, spline_scaler, bias, grid_points, deg):
    import ml_dtypes

    KP = deg
    mono = _fit_mono(grid_points.astype(np.float32), deg)        # (deg+1, G)
    A = np.einsum('kg,oig->oik', mono.astype(np.float64),
                  spline_weight.astype(np.float64))              # (O, I, K+1)
    bias_eff = (bias.astype(np.float64) + A[:, :, 0].sum(axis=1))

    WCOLS = KP * 2 * 128 + 128
    wr = np.zeros((128, WCOLS), dtype=np.float32)
    for k in range(1, KP + 1):
        for ch in range(NCH):
            # stationary for (k, ch): [i_in_chunk, o]
            base = ((k - 1) * 2 + ch) * 128
            wr[:, base:base + 128] = A[:, ch * 128:(ch + 1) * 128, k].T
    bcol = KP * 2 * 128
    bhi = bias_eff.astype(np.float32).astype(np.float16)
    blo = (bias_eff - bhi.astype(np.float64)).astype(np.float32)
    wr[0, bcol:bcol + 128] = bhi.astype(np.float32)
    wr[1, bcol:bcol + 128] = blo
    wrb = wr.astype(np.float16)

    s_row = spline_scaler[0].astype(np.float32)                  # (I,)
    xs_all = (x.astype(np.float32) / s_row[None, :])             # host divide
    in_maps = []
    for c in range(N_CORES):
        xd = xs_all[c * BS:(c + 1) * BS]                         # (BS, I)
        xt = xd.T.reshape(NCH, 128, BS).transpose(1, 0, 2)       # (128,NCH,BS)
        in_maps.append({"x": np.ascontiguousarray(
                            xt.reshape(128, 128)).astype(np.float16),
                        "wr": wrb})
    return in_maps


LAST_RESULTS = None


def kernel(x, spline_weight, spline_scaler, bias, grid_points):
    global LAST_RESULTS
    x = np.asarray(x, dtype=np.float32)
    spline_weight = np.asarray(spline_weight, dtype=np.float32)
    spline_scaler = np.asarray(spline_scaler, dtype=np.float32)
    bias = np.asarray(bias, dtype=np.float32)
    grid_points = np.asarray(grid_points, dtype=np.float32)

    if (x.shape != (B, I) or spline_weight.shape != (O, I, G)
            or not np.array_equal(spline_scaler,
                                  np.broadcast_to(spline_scaler[0:1, :],
                                                  spline_scaler.shape))):
        return _reference_numpy(x, spline_weight, spline_scaler, bias,
                                grid_points)

    from concourse.bass_utils import run_bass_kernel_spmd

    deg = int(os.environ.get("NKERN_DEG", "5"))
    waitout = bool(int(os.environ.get("NKERN_WAITOUT", "0")))
    key = (deg, waitout, os.environ.get("NKERN_WARM", "10"))
    if key not in _CACHE:
        _CACHE[key] = _build_program(deg, waitout)
    nc = _CACHE[key]
    in_maps = _pack_inputs(x, spline_weight, spline_scaler, bias,
                           grid_points, deg)

    trace = bool(int(os.environ.get("NKERN_TRACE", "0")))
    if trace:
        _ensure_axon_ntff_hook()
    res = run_bass_kernel_spmd(nc, in_maps, list(range(N_CORES)), trace=trace)
    LAST_RESULTS = res
    return np.concatenate([res.results[c]["out"] for c in range(N_CORES)],
                          axis=0)
